# revision 16
# baseline (speedup 1.0000x reference)
"""BiMambaBlock Trainium2 kernel (Bass/Tile), 8-core SPMD.

Sharding: core c -> (direction d=c//4, batch b=(c//2)%2, channel-half h=c%2).
Each core computes the full conv'd/silu'd u (needed for the x_proj
contraction over all of d_inner), then runs the selective scan, gate and
out_proj for its 1024-channel half only.  The feature-axis flip of the
reverse direction is folded into the in_proj weight slices on the host.
Partial out_proj results (each summed over the core's channel half) are
added together on the host: out[b] = sum over the 4 cores (2 dirs x 2
halves) of that batch.

On-chip layout is transposed: features on partitions, time on the free
axis.  The scan is the native DVE tensor_tensor_scan (h = dA*h + dBu along
t) run per (128-channel tile, state) pair.  dA = exp(delta * A[:,n]) is a
single ScalarE activation with per-partition scale.  B/C time-rows are
replicated across partitions via DRAM-bounce broadcast DMAs.  The state
reduction sum_n C*h is PE identity-matmul accumulation in PSUM.

build_bass(cfg, repeat=R) unrolls the whole compute body R times (inputs
loaded once) so true device time can be measured as (T(R)-T(1))/(R-1),
cancelling the multi-ms PJRT-over-axon dispatch overhead.
"""

import numpy as np

import concourse.bacc as bacc
import concourse.mybir as mybir
from concourse.bass_utils import run_bass_kernel_spmd
from concourse.tile import TileContext

f32 = mybir.dt.float32
f32r = mybir.dt.float32r
bf16 = mybir.dt.bfloat16
AF = mybir.ActivationFunctionType
OP = mybir.AluOpType

D_MODEL = 1024
D_INNER = 2048
HALF = D_INNER // 2          # channels per core
D_STATE = 16
D_CONV = 4
DT_RANK = 64
L = 1024
B = 2
TH = 512                     # matmul free-dim tile (time half)
P = 128
KD = D_MODEL // P            # 8 k-tiles over d_model
NU = D_INNER // P            # 16 u d-tiles (full)
NJ = HALF // P               # 8 own d-tiles
JBS = [2, 2, 2, 2]           # j-blocks (PSUM: 2j * 2h y banks = 4)

CFG = dict(
    dt_bc=bf16,       # B/C replicated rows
    dt_du=bf16,       # delta*u
    dt_da=bf16,       # exp(delta*A)
    dt_dbu=bf16,      # du*B
    dt_h=bf16,        # scan state output
    dt_hc=bf16,       # h*C
    dt_y2=bf16,       # gated output (out_proj rhs)
    dt_ident=bf16,    # identity lhsT for state reduction (match dt_hc)
    dt_wo=bf16,       # out_proj weights (match dt_y2)
    gp_hc=False,
    native_silu=True,
    native_softplus=False,
    pair_split=False,  # halve phase-1: own channel half only + AllReduce x_dbl
)


def build_bass(cfg=CFG, repeat=1):
    nc = bacc.Bacc(enable_partition_id=False,
                   num_devices=8 if cfg.get("pair_split") else None)

    xT = nc.declare_dram_parameter("xT", [D_MODEL, L], f32r, isOutput=False)
    wuz = nc.declare_dram_parameter("wuz", [P, (NU + NJ) * KD * P], f32r, isOutput=False)
    convd = nc.declare_dram_parameter("convd", [P, NU * D_CONV * P], f32r, isOutput=False)
    xpw = nc.declare_dram_parameter("xpw", [P, NU * 96], f32r, isOutput=False)
    dtw = nc.declare_dram_parameter("dtw", [DT_RANK, NJ * P], f32r, isOutput=False)
    wo = nc.declare_dram_parameter("wo", [P, KD * NJ * P], cfg["dt_wo"], isOutput=False)
    a_own = nc.declare_dram_parameter("a_own", [P, NJ * D_STATE], f32, isOutput=False)
    d_own = nc.declare_dram_parameter("d_own", [P, NJ], f32, isOutput=False)
    convb = nc.declare_dram_parameter("convb", [P, NU], f32, isOutput=False)
    dtb = nc.declare_dram_parameter("dtb", [P, NJ], f32, isOutput=False)
    ident_in = nc.declare_dram_parameter("ident", [P, P], cfg["dt_ident"], isOutput=False)
    outT = nc.declare_dram_parameter("outT", [D_MODEL, L], f32, isOutput=True)

    with TileContext(nc) as tc:
        with (
            tc.tile_pool(name="res", bufs=1) as res,
            tc.tile_pool(name="dramp", bufs=2, space="DRAM") as dramp,
        ):
            # resident tensors
            xt_sb = res.tile([P, KD * L], f32r, tag="xt")
            for k in range(KD):
                nc.sync.dma_start(out=xt_sb[:, k * L:(k + 1) * L],
                                  in_=xT[k * P:(k + 1) * P, :])
            id_sb = res.tile([P, P], cfg["dt_ident"], tag="id")
            nc.sync.dma_start(out=id_sb[:], in_=ident_in[:])
            a_sb = res.tile([P, NJ * D_STATE], f32, tag="a")
            nc.sync.dma_start(out=a_sb[:], in_=a_own[:])
            d_sb = res.tile([P, NJ], f32, tag="d")
            nc.sync.dma_start(out=d_sb[:], in_=d_own[:])
            cb_sb = res.tile([P, NU], f32, tag="cb")
            nc.sync.dma_start(out=cb_sb[:], in_=convb[:])
            dtb_sb = res.tile([P, NJ], f32, tag="dtb")
            nc.sync.dma_start(out=dtb_sb[:], in_=dtb[:])
            dtw_sb = res.tile([DT_RANK, NJ * P], f32r, tag="dtw")
            nc.sync.dma_start(out=dtw_sb[:], in_=dtw[:])
            xpw_sb = res.tile([P, NU * 96], f32r, tag="xpw")
            nc.sync.dma_start(out=xpw_sb[:], in_=xpw[:])
            u_own = res.tile([P, NJ * L], bf16, tag="uown")
            y2_sb = res.tile([P, NJ * L], cfg["dt_y2"], tag="y2")
            xdbl_sb = res.tile([96, L], f32r, tag="xdbl")
            delta_sb = res.tile([P, NJ * L], bf16, tag="delta")
            scr_sb = res.tile([P, NJ * L], bf16, tag="scr")

            for rep in range(repeat):
                _body(nc, tc, cfg, rep, wuz, convd, dtw, wo, outT,
                      xt_sb, id_sb, a_sb, d_sb, cb_sb, dtb_sb, dtw_sb, xpw_sb,
                      u_own, y2_sb, xdbl_sb, delta_sb, scr_sb, dramp)
    nc.compile()
    return nc


def _body(nc, tc, cfg, rep, wuz, convd, dtw, wo, outT,
          xt_sb, id_sb, a_sb, d_sb, cb_sb, dtb_sb, dtw_sb, xpw_sb,
          u_own, y2_sb, xdbl_sb, delta_sb, scr_sb, dramp):
    bc_dram = dramp.tile([2 * D_STATE, L], cfg["dt_bc"], tag="bc",
                         name=f"bc{rep}")
    # ---------------- phase 1: in_proj(u) + conv + silu + x_proj ----
    with (
        tc.tile_pool(name="xdblp", bufs=1, space="PSUM") as xdblp,
    ):
        xdbl_ps = [xdblp.tile([96, TH], f32, tag=f"xd{h}", name=f"xdbl_ps{h}")
                   for h in range(2)]
        nu_l = NJ if cfg.get("pair_split") else NU
        with (
            tc.tile_pool(name="wp", bufs=3) as wp,
            tc.tile_pool(name="cp", bufs=3) as cp,
            tc.tile_pool(name="upre", bufs=4) as upre_p,
            tc.tile_pool(name="up", bufs=3) as up,
            tc.tile_pool(name="tmp1", bufs=6) as tmp1,
            tc.tile_pool(name="mmps", bufs=3, space="PSUM") as mmps,
        ):
            for m in range(nu_l):
                w_sb = wp.tile([P, KD * P], f32r, tag="w")
                nc.sync.dma_start(out=w_sb[:],
                                  in_=wuz[:, m * KD * P:(m + 1) * KD * P])
                ps = [mmps.tile([P, TH], f32, tag=f"mm{h}", name=f"ps{h}")
                      for h in range(2)]
                for h in range(2):
                    for k in range(KD):
                        nc.tensor.matmul(
                            ps[h][:], w_sb[:, k * P:(k + 1) * P],
                            xt_sb[:, k * L + h * TH: k * L + (h + 1) * TH],
                            start=(k == 0), stop=(k == KD - 1))
                upre = upre_p.tile([P, L + D_CONV - 1], f32r, tag="upre")
                nc.vector.memset(upre[:, 0:D_CONV - 1].bitcast(f32), 0.0)
                for h in range(2):
                    nc.scalar.activation(
                        upre[:, D_CONV - 1 + h * TH: D_CONV - 1 + (h + 1) * TH],
                        ps[h][:], AF.Copy)
                cd = cp.tile([P, D_CONV * P], f32r, tag="cd")
                nc.sync.dma_start(
                    out=cd[:], in_=convd[:, m * D_CONV * P:(m + 1) * D_CONV * P])
                psc = [mmps.tile([P, TH], f32, tag=f"mm{h}", name=f"psc{h}")
                       for h in range(2)]
                for h in range(2):
                    for kk in range(D_CONV):
                        nc.tensor.matmul(
                            psc[h][:], cd[:, kk * P:(kk + 1) * P],
                            upre[:, kk + h * TH: kk + h * TH + TH],
                            start=(kk == 0), stop=(kk == D_CONV - 1))
                u_m = up.tile([P, L], f32r, tag="u")
                for h in range(2):
                    sl = slice(h * TH, (h + 1) * TH)
                    if cfg["native_silu"]:
                        nc.scalar.activation(u_m[:, sl], psc[h][:], AF.Silu,
                                             bias=cb_sb[:, m:m + 1])
                    else:
                        sg = tmp1.tile([P, TH], f32, tag="sg")
                        uc = tmp1.tile([P, TH], f32, tag="uc")
                        nc.scalar.activation(sg[:], psc[h][:], AF.Sigmoid,
                                             bias=cb_sb[:, m:m + 1])
                        nc.scalar.activation(uc[:], psc[h][:], AF.Identity,
                                             bias=cb_sb[:, m:m + 1])
                        nc.vector.tensor_mul(u_m[:, sl], uc[:], sg[:])
                    nc.tensor.matmul(
                        xdbl_ps[h][:], xpw_sb[:, m * 96:(m + 1) * 96],
                        u_m[:, sl], start=(m == 0), stop=(m == nu_l - 1))
                if m < NJ:  # own half: keep bf16 copy for du and skip path
                    nc.scalar.activation(u_own[:, m * L:(m + 1) * L], u_m[:],
                                         AF.Copy)

        # ---------------- phase 2: drain x_dbl, stage B/C --------------
        if cfg.get("pair_split"):
            # partial x_dbl -> DRAM -> AllReduce(add) with pair core ->
            # back into xdbl_sb.
            xp_dram = dramp.tile([96, L], f32, tag="xp", name=f"xp{rep}")
            xr_dram = dramp.tile([96, L], f32, tag="xr", name=f"xr{rep}")
            with tc.tile_pool(name="xpart", bufs=1) as xpp:
                xpart = xpp.tile([96, L], f32, tag="xpart")
                for h in range(2):
                    nc.vector.tensor_copy(xpart[:, h * TH:(h + 1) * TH],
                                          xdbl_ps[h][:])
                nc.sync.dma_start(out=xp_dram[:], in_=xpart[:])
            nc.gpsimd.collective_compute(
                "AllReduce", OP.add,
                replica_groups=[[0, 1], [2, 3], [4, 5], [6, 7]],
                ins=[xp_dram[:].opt()], outs=[xr_dram[:].opt()])
            nc.sync.dma_start(out=xdbl_sb[:].bitcast(f32), in_=xr_dram[:])
        else:
            for h in range(2):
                nc.vector.tensor_copy(xdbl_sb[:, h * TH:(h + 1) * TH],
                                      xdbl_ps[h][:])
        if cfg["dt_bc"] == f32:
            nc.sync.dma_start(out=bc_dram[:], in_=xdbl_sb[DT_RANK:96, :])
        else:
            with tc.tile_pool(name="bcc", bufs=1) as bcc:
                bc_cast = bcc.tile([2 * D_STATE, L], cfg["dt_bc"], tag="bcc")
                nc.vector.tensor_copy(bc_cast[:], xdbl_sb[DT_RANK:96, :])
                nc.sync.dma_start(out=bc_dram[:], in_=bc_cast[:])

    # ---------------- phase 3: scan (delta/z/gate interleaved) ------
    # Per j-block: the block's delta (dt-proj + softplus) was computed at
    # the previous block boundary (or in the prologue for block 0), so the
    # Act engine computes the NEXT block's delta and this block's z-gate
    # while the DVE chews through the scan.  PSUM: 4 y + 2 z + 2 dt = 8.
    def _delta_block(dtps, js):
        for j in js:
            for h in range(2):
                dps = dtps.tile([P, TH], f32, tag="dt", name=f"dps{j}{h}")
                nc.tensor.matmul(
                    dps[:], dtw_sb[:, j * P:(j + 1) * P],
                    xdbl_sb[0:DT_RANK, h * TH:(h + 1) * TH],
                    start=True, stop=True)
                sl = slice(j * L + h * TH, j * L + (h + 1) * TH)
                if cfg["native_softplus"]:
                    nc.scalar.activation(delta_sb[:, sl], dps[:],
                                         AF.Softplus, bias=dtb_sb[:, j:j + 1])
                else:
                    nc.scalar.activation(scr_sb[:, sl], dps[:], AF.Exp,
                                         bias=dtb_sb[:, j:j + 1])
        if not cfg["native_softplus"]:
            for j in js:
                sl = slice(j * L, (j + 1) * L)
                nc.scalar.activation(delta_sb[:, sl], scr_sb[:, sl], AF.Ln,
                                     bias=1.0)

    with (
        tc.tile_pool(name="dup", bufs=4) as dup,
        tc.tile_pool(name="bcp", bufs=4) as bcp,
        tc.tile_pool(name="sda", bufs=4) as sda,
        tc.tile_pool(name="sdb", bufs=4) as sdb,
        tc.tile_pool(name="shp", bufs=4) as shp,
        tc.tile_pool(name="shc", bufs=4) as shc,
        tc.tile_pool(name="tmp3", bufs=4) as tmp3,
        tc.tile_pool(name="wzp", bufs=2) as wzp,
        tc.tile_pool(name="dtps", bufs=2, space="PSUM") as dtps,
        tc.tile_pool(name="yps", bufs=1, space="PSUM") as yps,
        tc.tile_pool(name="zpsp", bufs=1, space="PSUM") as zpsp,
    ):
        blocks = []
        j0 = 0
        for jb in JBS:
            blocks.append(list(range(j0, j0 + jb)))
            j0 += jb
        _delta_block(dtps, blocks[0])

        for b, js in enumerate(blocks):
            dus = {}
            for j in js:
                du_j = dup.tile([P, L], cfg["dt_du"], tag="du")
                nc.vector.tensor_mul(du_j[:], delta_sb[:, j * L:(j + 1) * L],
                                     u_own[:, j * L:(j + 1) * L])
                dus[j] = du_j

            y_ps = {j: [yps.tile([P, TH], f32, tag=f"y{j - js[0]}{h}",
                                 name=f"yps{j}{h}")
                        for h in range(2)] for j in js}
            for n in range(D_STATE):
                brep = bcp.tile([P, L], cfg["dt_bc"], tag="brep")
                nc.sync.dma_start(out=brep[:],
                                  in_=bc_dram[n, :].partition_broadcast(P))
                crep = bcp.tile([P, L], cfg["dt_bc"], tag="crep")
                nc.sync.dma_start(
                    out=crep[:],
                    in_=bc_dram[D_STATE + n, :].partition_broadcast(P))
                for j in js:
                    da = sda.tile([P, L], cfg["dt_da"], tag="da")
                    nc.scalar.activation(
                        da[:], delta_sb[:, j * L:(j + 1) * L], AF.Exp,
                        scale=a_sb[:, j * D_STATE + n: j * D_STATE + n + 1])
                    dbu = sdb.tile([P, L], cfg["dt_dbu"], tag="dbu")
                    nc.vector.tensor_mul(dbu[:], dus[j][:], brep[:])
                    h_t = shp.tile([P, L], cfg["dt_h"], tag="h")
                    nc.vector.tensor_tensor_scan(h_t[:], da[:], dbu[:], 0.0,
                                                 OP.mult, OP.add)
                    hc = shc.tile([P, L], cfg["dt_hc"], tag="hc")
                    nc.vector.tensor_mul(hc[:], h_t[:], crep[:])
                    for h in range(2):
                        nc.tensor.matmul(
                            y_ps[j][h][:], id_sb[:],
                            hc[:, h * TH:(h + 1) * TH],
                            start=(n == 0), stop=(n == D_STATE - 1))
                if n == 0 and b + 1 < len(blocks):
                    _delta_block(dtps, blocks[b + 1])

            for j in js:
                # y1 = u_own * D + y  -> scr_sb (exp scratch is free now)
                for h in range(2):
                    nc.vector.scalar_tensor_tensor(
                        scr_sb[:, j * L + h * TH: j * L + (h + 1) * TH],
                        u_own[:, j * L + h * TH: j * L + (h + 1) * TH],
                        d_sb[:, j:j + 1], y_ps[j][h][:], OP.mult, OP.add)
                # z-proj + gate: y2 = y1 * silu(z)
                zw = wzp.tile([P, KD * P], f32r, tag="zw")
                nc.sync.dma_start(
                    out=zw[:],
                    in_=wuz[:, (NU + j) * KD * P:(NU + j + 1) * KD * P])
                zps = [zpsp.tile([P, TH], f32, tag=f"zp{h}", name=f"zps{j}{h}")
                       for h in range(2)]
                for h in range(2):
                    for k in range(KD):
                        nc.tensor.matmul(
                            zps[h][:], zw[:, k * P:(k + 1) * P],
                            xt_sb[:, k * L + h * TH: k * L + (h + 1) * TH],
                            start=(k == 0), stop=(k == KD - 1))
                for h in range(2):
                    ysl = slice(j * L + h * TH, j * L + (h + 1) * TH)
                    if cfg["native_silu"]:
                        sz = tmp3.tile([P, TH], bf16, tag="sz")
                        nc.scalar.activation(sz[:], zps[h][:], AF.Silu)
                        nc.vector.tensor_mul(y2_sb[:, ysl], scr_sb[:, ysl],
                                             sz[:])
                    else:
                        sg = tmp3.tile([P, TH], f32, tag="sz")
                        t1 = tmp3.tile([P, TH], f32, tag="t1")
                        nc.scalar.activation(sg[:], zps[h][:], AF.Sigmoid)
                        nc.vector.tensor_mul(t1[:], scr_sb[:, ysl], sg[:])
                        nc.vector.tensor_mul(y2_sb[:, ysl], t1[:], zps[h][:])

    # ---------------- phase 4: out_proj -----------------------------
    with (
        tc.tile_pool(name="wop", bufs=2) as wop,
        tc.tile_pool(name="osbp", bufs=4) as osbp,
        tc.tile_pool(name="ops", bufs=2, space="PSUM") as ops,
    ):
        for m in range(KD):
            wo_sb = wop.tile([P, NJ * P], cfg["dt_wo"], tag="wo")
            nc.sync.dma_start(out=wo_sb[:],
                              in_=wo[:, m * NJ * P:(m + 1) * NJ * P])
            po = [ops.tile([P, TH], f32, tag=f"po{h}", name=f"po{h}")
                  for h in range(2)]
            for h in range(2):
                for k in range(NJ):
                    nc.tensor.matmul(
                        po[h][:], wo_sb[:, k * P:(k + 1) * P],
                        y2_sb[:, k * L + h * TH: k * L + (h + 1) * TH],
                        start=(k == 0), stop=(k == NJ - 1))
                osb = osbp.tile([P, TH], f32, tag="osb", name=f"osb{m}{h}")
                nc.scalar.activation(osb[:], po[h][:], AF.Copy)
                nc.sync.dma_start(
                    out=outT[m * P:(m + 1) * P, h * TH:(h + 1) * TH],
                    in_=osb[:])


def _np(x):
    return np.asarray(x, dtype=np.float32)


def _round_f32r(x):
    """Round-to-nearest-even at 11 mantissa bits (matches HW f32r)."""
    i = np.ascontiguousarray(x, np.float32).view(np.uint32).astype(np.uint64)
    shift = 23 - 11
    bias = ((i >> shift) & 1) + ((1 << (shift - 1)) - 1)
    return ((i + bias) >> shift << shift).astype(np.uint32).view(np.float32)


def pack_core(c, inp, cfg=CFG):
    """Build the input map for core c from the full-problem inputs."""
    d, b, half = c // 4, (c // 2) % 2, c % 2
    tag = "f" if d == 0 else "r"
    w_in = _np(inp["in_proj_w"])
    if d == 1:
        w_in = w_in[:, ::-1]
    w_u, w_z = w_in[:D_INNER], w_in[D_INNER:]
    conv_w = _np(inp[f"conv_w_{tag}"])[:, 0, :]          # [D_INNER, 4]
    conv_b = _np(inp[f"conv_b_{tag}"])
    x_proj_w = _np(inp[f"x_proj_w_{tag}"])               # [96, D_INNER]
    dt_w = _np(inp[f"dt_w_{tag}"])                       # [D_INNER, 64]
    dt_b = _np(inp[f"dt_b_{tag}"])
    a_full = -np.exp(_np(inp[f"A_log_{tag}"]))           # [D_INNER, 16]
    d_full = _np(inp[f"D_{tag}"])
    w_out = _np(inp["out_proj_w"])                       # [1024, D_INNER]

    x = _np(inp["x"])[b] * _np(inp["input_mask"])[b]     # [L, 1024]
    own = half * HALF
    oth = (1 - half) * HALF

    # u-tile channel order: own half first
    uch = [own + m * P for m in range(NJ)] + [oth + m * P for m in range(NJ)]

    wuz = np.empty((P, (NU + NJ) * KD * P), np.float32)
    convd = np.zeros((P, NU * D_CONV * P), np.float32)
    convb_a = np.empty((P, NU), np.float32)
    xpw_a = np.empty((P, NU * 96), np.float32)
    for m, ch in enumerate(uch):
        for k in range(KD):
            wuz[:, m * KD * P + k * P:(m * KD + k + 1) * P] = \
                w_u[ch:ch + P, k * P:(k + 1) * P].T
        for kk in range(D_CONV):
            blk = convd[:, (m * D_CONV + kk) * P:(m * D_CONV + kk + 1) * P]
            np.fill_diagonal(blk, conv_w[ch:ch + P, kk])
        convb_a[:, m] = conv_b[ch:ch + P]
        xpw_a[:, m * 96:(m + 1) * 96] = x_proj_w[:, ch:ch + P].T
    for j in range(NJ):
        ch = own + j * P
        for k in range(KD):
            wuz[:, ((NU + j) * KD + k) * P:((NU + j) * KD + k + 1) * P] = \
                w_z[ch:ch + P, k * P:(k + 1) * P].T

    dtw_a = np.empty((DT_RANK, NJ * P), np.float32)
    dtb_a = np.empty((P, NJ), np.float32)
    a_own = np.empty((P, NJ * D_STATE), np.float32)
    d_own = np.empty((P, NJ), np.float32)
    wo_a = np.empty((P, KD * NJ * P), np.float32)
    for j in range(NJ):
        ch = own + j * P
        dtw_a[:, j * P:(j + 1) * P] = dt_w[ch:ch + P, :].T
        dtb_a[:, j] = dt_b[ch:ch + P]
        a_own[:, j * D_STATE:(j + 1) * D_STATE] = a_full[ch:ch + P]
        d_own[:, j] = d_full[ch:ch + P]
    for m in range(KD):
        for k in range(NJ):
            wo_a[:, (m * NJ + k) * P:(m * NJ + k + 1) * P] = \
                w_out[m * P:(m + 1) * P, own + k * P: own + (k + 1) * P].T

    import ml_dtypes
    wo_cast = wo_a.astype(np.float32 if cfg["dt_wo"] == f32 else ml_dtypes.bfloat16)
    id_np = np.eye(P).astype(np.float32 if cfg["dt_ident"] == f32 else ml_dtypes.bfloat16)

    return dict(
        xT=_round_f32r(np.ascontiguousarray(x.T)),
        wuz=_round_f32r(wuz), convd=_round_f32r(convd),
        xpw=_round_f32r(xpw_a), dtw=_round_f32r(dtw_a), wo=wo_cast,
        a_own=a_own, d_own=d_own, convb=convb_a, dtb=dtb_a,
        ident=id_np,
    )


_NC_CACHE = {}


def _get_nc(repeat=1):
    if repeat not in _NC_CACHE:
        _NC_CACHE[repeat] = build_bass(CFG, repeat=repeat)
    return _NC_CACHE[repeat]


def kernel(**inputs):
    nc = _get_nc()
    in_maps = [pack_core(c, inputs) for c in range(8)]
    res = run_bass_kernel_spmd(nc, in_maps, core_ids=list(range(8)))
    out = np.zeros((B, L, D_MODEL), np.float32)
    for c in range(8):
        b = (c // 2) % 2
        out[b] += np.asarray(res.results[c]["outT"], np.float32).T
    return out



# revision 17
# speedup vs baseline: 1.0199x; 1.0199x over previous
"""BiMambaBlock Trainium2 kernel (Bass/Tile), 8-core SPMD.

Sharding: core c -> (direction d=c//4, batch b=(c//2)%2, channel-half h=c%2).
Each core computes the full conv'd/silu'd u (needed for the x_proj
contraction over all of d_inner), then runs the selective scan, gate and
out_proj for its 1024-channel half only.  The feature-axis flip of the
reverse direction is folded into the in_proj weight slices on the host.
Partial out_proj results (each summed over the core's channel half) are
added together on the host: out[b] = sum over the 4 cores (2 dirs x 2
halves) of that batch.

On-chip layout is transposed: features on partitions, time on the free
axis.  The scan is the native DVE tensor_tensor_scan (h = dA*h + dBu along
t) run per (128-channel tile, state) pair.  dA = exp(delta * A[:,n]) is a
single ScalarE activation with per-partition scale.  B/C time-rows are
replicated across partitions via DRAM-bounce broadcast DMAs.  The state
reduction sum_n C*h is PE identity-matmul accumulation in PSUM.

build_bass(cfg, repeat=R) unrolls the whole compute body R times (inputs
loaded once) so true device time can be measured as (T(R)-T(1))/(R-1),
cancelling the multi-ms PJRT-over-axon dispatch overhead.
"""

import numpy as np

import concourse.bacc as bacc
import concourse.mybir as mybir
from concourse.bass_utils import run_bass_kernel_spmd
from concourse.tile import TileContext

f32 = mybir.dt.float32
f32r = mybir.dt.float32r
bf16 = mybir.dt.bfloat16
AF = mybir.ActivationFunctionType
OP = mybir.AluOpType

D_MODEL = 1024
D_INNER = 2048
HALF = D_INNER // 2          # channels per core
D_STATE = 16
D_CONV = 4
DT_RANK = 64
L = 1024
B = 2
TH = 512                     # matmul free-dim tile (time half)
P = 128
KD = D_MODEL // P            # 8 k-tiles over d_model
NU = D_INNER // P            # 16 u d-tiles (full)
NJ = HALF // P               # 8 own d-tiles
JBS = [2, 2, 2, 2]           # j-blocks (PSUM: 2j * 2h y banks = 4)

CFG = dict(
    dt_bc=bf16,       # B/C replicated rows
    dt_du=bf16,       # delta*u
    dt_da=bf16,       # exp(delta*A)
    dt_dbu=bf16,      # du*B
    dt_h=bf16,        # scan state output
    dt_hc=bf16,       # h*C
    dt_y2=bf16,       # gated output (out_proj rhs)
    dt_ident=bf16,    # identity lhsT for state reduction (match dt_hc)
    dt_wo=bf16,       # out_proj weights (match dt_y2)
    gp_hc=False,
    native_silu=True,
    native_softplus=False,
    pair_split=False,  # halve phase-1: own channel half only + AllReduce x_dbl
)


def build_bass(cfg=CFG, repeat=1):
    nc = bacc.Bacc(enable_partition_id=False,
                   num_devices=8 if cfg.get("pair_split") else None)

    xT = nc.declare_dram_parameter("xT", [D_MODEL, L], f32r, isOutput=False)
    wuz = nc.declare_dram_parameter("wuz", [P, (NU + NJ) * KD * P], f32r, isOutput=False)
    convd = nc.declare_dram_parameter("convd", [P, NU * D_CONV * P], f32r, isOutput=False)
    xpw = nc.declare_dram_parameter("xpw", [P, NU * 96], f32r, isOutput=False)
    dtw = nc.declare_dram_parameter("dtw", [DT_RANK, NJ * P], f32r, isOutput=False)
    wo = nc.declare_dram_parameter("wo", [P, KD * NJ * P], cfg["dt_wo"], isOutput=False)
    a_own = nc.declare_dram_parameter("a_own", [P, NJ * D_STATE], f32, isOutput=False)
    d_own = nc.declare_dram_parameter("d_own", [P, NJ], f32, isOutput=False)
    convb = nc.declare_dram_parameter("convb", [P, NU], f32, isOutput=False)
    dtb = nc.declare_dram_parameter("dtb", [P, NJ], f32, isOutput=False)
    ident_in = nc.declare_dram_parameter("ident", [P, P], cfg["dt_ident"], isOutput=False)
    outT = nc.declare_dram_parameter("outT", [D_MODEL, L], f32, isOutput=True)

    with TileContext(nc) as tc:
        with (
            tc.tile_pool(name="res", bufs=1) as res,
            tc.tile_pool(name="dramp", bufs=2, space="DRAM") as dramp,
        ):
            # resident tensors
            xt_sb = res.tile([P, KD * L], f32r, tag="xt")
            for k in range(KD):
                nc.sync.dma_start(out=xt_sb[:, k * L:(k + 1) * L],
                                  in_=xT[k * P:(k + 1) * P, :])
            id_sb = res.tile([P, P], cfg["dt_ident"], tag="id")
            nc.sync.dma_start(out=id_sb[:], in_=ident_in[:])
            a_sb = res.tile([P, NJ * D_STATE], f32, tag="a")
            nc.sync.dma_start(out=a_sb[:], in_=a_own[:])
            d_sb = res.tile([P, NJ], f32, tag="d")
            nc.sync.dma_start(out=d_sb[:], in_=d_own[:])
            cb_sb = res.tile([P, NU], f32, tag="cb")
            nc.sync.dma_start(out=cb_sb[:], in_=convb[:])
            dtb_sb = res.tile([P, NJ], f32, tag="dtb")
            nc.sync.dma_start(out=dtb_sb[:], in_=dtb[:])
            dtw_sb = res.tile([DT_RANK, NJ * P], f32r, tag="dtw")
            nc.sync.dma_start(out=dtw_sb[:], in_=dtw[:])
            xpw_sb = res.tile([P, NU * 96], f32r, tag="xpw")
            nc.sync.dma_start(out=xpw_sb[:], in_=xpw[:])
            u_own = res.tile([P, NJ * L], bf16, tag="uown")
            y2_sb = res.tile([P, NJ * L], cfg["dt_y2"], tag="y2")
            xdbl_sb = res.tile([96, L], f32r, tag="xdbl")
            delta_sb = res.tile([P, NJ * L], bf16, tag="delta")
            scr_sb = res.tile([P, NJ * L], bf16, tag="scr")

            for rep in range(repeat):
                _body(nc, tc, cfg, rep, wuz, convd, dtw, wo, outT,
                      xt_sb, id_sb, a_sb, d_sb, cb_sb, dtb_sb, dtw_sb, xpw_sb,
                      u_own, y2_sb, xdbl_sb, delta_sb, scr_sb, dramp)
    nc.compile()
    return nc


def _body(nc, tc, cfg, rep, wuz, convd, dtw, wo, outT,
          xt_sb, id_sb, a_sb, d_sb, cb_sb, dtb_sb, dtw_sb, xpw_sb,
          u_own, y2_sb, xdbl_sb, delta_sb, scr_sb, dramp):
    bc_dram = dramp.tile([2 * D_STATE, L], cfg["dt_bc"], tag="bc",
                         name=f"bc{rep}")
    # ---------------- phase 1: in_proj(u) + conv + silu + x_proj ----
    with (
        tc.tile_pool(name="xdblp", bufs=1, space="PSUM") as xdblp,
    ):
        xdbl_ps = [xdblp.tile([96, TH], f32, tag=f"xd{h}", name=f"xdbl_ps{h}")
                   for h in range(2)]
        nu_l = NJ if cfg.get("pair_split") else NU
        with (
            tc.tile_pool(name="wp", bufs=3) as wp,
            tc.tile_pool(name="cp", bufs=3) as cp,
            tc.tile_pool(name="upre", bufs=4) as upre_p,
            tc.tile_pool(name="up", bufs=3) as up,
            tc.tile_pool(name="tmp1", bufs=6) as tmp1,
            tc.tile_pool(name="mmps", bufs=3, space="PSUM") as mmps,
        ):
            for m in range(nu_l):
                w_sb = wp.tile([P, KD * P], f32r, tag="w")
                nc.sync.dma_start(out=w_sb[:],
                                  in_=wuz[:, m * KD * P:(m + 1) * KD * P])
                ps = [mmps.tile([P, TH], f32, tag=f"mm{h}", name=f"ps{h}")
                      for h in range(2)]
                for h in range(2):
                    for k in range(KD):
                        nc.tensor.matmul(
                            ps[h][:], w_sb[:, k * P:(k + 1) * P],
                            xt_sb[:, k * L + h * TH: k * L + (h + 1) * TH],
                            start=(k == 0), stop=(k == KD - 1))
                upre = upre_p.tile([P, L + D_CONV - 1], f32r, tag="upre")
                nc.vector.memset(upre[:, 0:D_CONV - 1].bitcast(f32), 0.0)
                for h in range(2):
                    nc.scalar.activation(
                        upre[:, D_CONV - 1 + h * TH: D_CONV - 1 + (h + 1) * TH],
                        ps[h][:], AF.Copy)
                cd = cp.tile([P, D_CONV * P], f32r, tag="cd")
                nc.sync.dma_start(
                    out=cd[:], in_=convd[:, m * D_CONV * P:(m + 1) * D_CONV * P])
                psc = [mmps.tile([P, TH], f32, tag=f"mm{h}", name=f"psc{h}")
                       for h in range(2)]
                for h in range(2):
                    for kk in range(D_CONV):
                        nc.tensor.matmul(
                            psc[h][:], cd[:, kk * P:(kk + 1) * P],
                            upre[:, kk + h * TH: kk + h * TH + TH],
                            start=(kk == 0), stop=(kk == D_CONV - 1))
                u_m = up.tile([P, L], f32r, tag="u")
                for h in range(2):
                    sl = slice(h * TH, (h + 1) * TH)
                    if cfg["native_silu"]:
                        nc.scalar.activation(u_m[:, sl], psc[h][:], AF.Silu,
                                             bias=cb_sb[:, m:m + 1])
                    else:
                        sg = tmp1.tile([P, TH], f32, tag="sg")
                        uc = tmp1.tile([P, TH], f32, tag="uc")
                        nc.scalar.activation(sg[:], psc[h][:], AF.Sigmoid,
                                             bias=cb_sb[:, m:m + 1])
                        nc.scalar.activation(uc[:], psc[h][:], AF.Identity,
                                             bias=cb_sb[:, m:m + 1])
                        nc.vector.tensor_mul(u_m[:, sl], uc[:], sg[:])
                    nc.tensor.matmul(
                        xdbl_ps[h][:], xpw_sb[:, m * 96:(m + 1) * 96],
                        u_m[:, sl], start=(m == 0), stop=(m == nu_l - 1))
                if m < NJ:  # own half: keep bf16 copy for du and skip path
                    nc.scalar.activation(u_own[:, m * L:(m + 1) * L], u_m[:],
                                         AF.Copy)

        # ---------------- phase 2: drain x_dbl, stage B/C --------------
        if cfg.get("pair_split"):
            # partial x_dbl -> DRAM -> AllReduce(add) with pair core ->
            # back into xdbl_sb.
            xp_dram = dramp.tile([96, L], f32, tag="xp", name=f"xp{rep}")
            xr_dram = dramp.tile([96, L], f32, tag="xr", name=f"xr{rep}")
            with tc.tile_pool(name="xpart", bufs=1) as xpp:
                xpart = xpp.tile([96, L], f32, tag="xpart")
                for h in range(2):
                    nc.vector.tensor_copy(xpart[:, h * TH:(h + 1) * TH],
                                          xdbl_ps[h][:])
                nc.sync.dma_start(out=xp_dram[:], in_=xpart[:])
            nc.gpsimd.collective_compute(
                "AllReduce", OP.add,
                replica_groups=[[0, 1], [2, 3], [4, 5], [6, 7]],
                ins=[xp_dram[:].opt()], outs=[xr_dram[:].opt()])
            nc.sync.dma_start(out=xdbl_sb[:].bitcast(f32), in_=xr_dram[:])
        else:
            for h in range(2):
                nc.vector.tensor_copy(xdbl_sb[:, h * TH:(h + 1) * TH],
                                      xdbl_ps[h][:])
        if cfg["dt_bc"] == f32:
            nc.sync.dma_start(out=bc_dram[:], in_=xdbl_sb[DT_RANK:96, :])
        else:
            with tc.tile_pool(name="bcc", bufs=1) as bcc:
                bc_cast = bcc.tile([2 * D_STATE, L], cfg["dt_bc"], tag="bcc")
                nc.vector.tensor_copy(bc_cast[:], xdbl_sb[DT_RANK:96, :])
                nc.sync.dma_start(out=bc_dram[:], in_=bc_cast[:])

    # ---------------- phase 3: scan (delta/z/gate interleaved) ------
    # Per j-block: the block's delta (dt-proj + softplus) was computed at
    # the previous block boundary (or in the prologue for block 0), so the
    # Act engine computes the NEXT block's delta and this block's z-gate
    # while the DVE chews through the scan.  PSUM: 4 y + 2 z + 2 dt = 8.
    def _delta_block(dtps, js):
        for j in js:
            for h in range(2):
                dps = dtps.tile([P, TH], f32, tag="dt", name=f"dps{j}{h}")
                nc.tensor.matmul(
                    dps[:], dtw_sb[:, j * P:(j + 1) * P],
                    xdbl_sb[0:DT_RANK, h * TH:(h + 1) * TH],
                    start=True, stop=True)
                sl = slice(j * L + h * TH, j * L + (h + 1) * TH)
                if cfg["native_softplus"]:
                    nc.scalar.activation(delta_sb[:, sl], dps[:],
                                         AF.Softplus, bias=dtb_sb[:, j:j + 1])
                else:
                    nc.scalar.activation(scr_sb[:, sl], dps[:], AF.Exp,
                                         bias=dtb_sb[:, j:j + 1])
        if not cfg["native_softplus"]:
            for j in js:
                sl = slice(j * L, (j + 1) * L)
                nc.scalar.activation(delta_sb[:, sl], scr_sb[:, sl], AF.Ln,
                                     bias=1.0)

    with (
        tc.tile_pool(name="dup", bufs=3) as dup,
        tc.tile_pool(name="bcp", bufs=3) as bcp,
        tc.tile_pool(name="sda", bufs=3) as sda,
        tc.tile_pool(name="sdb", bufs=3) as sdb,
        tc.tile_pool(name="shp", bufs=3) as shp,
        tc.tile_pool(name="shc", bufs=3) as shc,
        tc.tile_pool(name="tmp3", bufs=4) as tmp3,
        tc.tile_pool(name="wzp", bufs=2) as wzp,
        tc.tile_pool(name="dtps", bufs=2, space="PSUM") as dtps,
        tc.tile_pool(name="yps", bufs=1, space="PSUM") as yps,
        tc.tile_pool(name="zpsp", bufs=1, space="PSUM") as zpsp,
    ):
        blocks = []
        j0 = 0
        for jb in JBS:
            blocks.append(list(range(j0, j0 + jb)))
            j0 += jb
        _delta_block(dtps, blocks[0])

        for b, js in enumerate(blocks):
            dus = {}
            for j in js:
                du_j = dup.tile([P, L], cfg["dt_du"], tag="du")
                nc.vector.tensor_mul(du_j[:], delta_sb[:, j * L:(j + 1) * L],
                                     u_own[:, j * L:(j + 1) * L])
                dus[j] = du_j

            y_ps = {j: [yps.tile([P, TH], f32, tag=f"y{j - js[0]}{h}",
                                 name=f"yps{j}{h}")
                        for h in range(2)] for j in js}
            for n in range(D_STATE):
                brep = bcp.tile([P, L], cfg["dt_bc"], tag="brep")
                nc.sync.dma_start(out=brep[:],
                                  in_=bc_dram[n, :].partition_broadcast(P))
                crep = bcp.tile([P, L], cfg["dt_bc"], tag="crep")
                nc.sync.dma_start(
                    out=crep[:],
                    in_=bc_dram[D_STATE + n, :].partition_broadcast(P))
                for j in js:
                    da = sda.tile([P, L], cfg["dt_da"], tag="da")
                    nc.scalar.activation(
                        da[:], delta_sb[:, j * L:(j + 1) * L], AF.Exp,
                        scale=a_sb[:, j * D_STATE + n: j * D_STATE + n + 1])
                    dbu = sdb.tile([P, L], cfg["dt_dbu"], tag="dbu")
                    nc.vector.tensor_mul(dbu[:], dus[j][:], brep[:])
                    h_t = shp.tile([P, L], cfg["dt_h"], tag="h")
                    nc.vector.tensor_tensor_scan(h_t[:], da[:], dbu[:], 0.0,
                                                 OP.mult, OP.add)
                    hc = shc.tile([P, L], cfg["dt_hc"], tag="hc")
                    nc.vector.tensor_mul(hc[:], h_t[:], crep[:])
                    for h in range(2):
                        nc.tensor.matmul(
                            y_ps[j][h][:], id_sb[:],
                            hc[:, h * TH:(h + 1) * TH],
                            start=(n == 0), stop=(n == D_STATE - 1))
                if n == 0 and b + 1 < len(blocks):
                    _delta_block(dtps, blocks[b + 1])

            for j in js:
                # y1 = u_own * D + y  -> scr_sb (exp scratch is free now)
                for h in range(2):
                    nc.vector.scalar_tensor_tensor(
                        scr_sb[:, j * L + h * TH: j * L + (h + 1) * TH],
                        u_own[:, j * L + h * TH: j * L + (h + 1) * TH],
                        d_sb[:, j:j + 1], y_ps[j][h][:], OP.mult, OP.add)
                # z-proj + gate: y2 = y1 * silu(z)
                zw = wzp.tile([P, KD * P], f32r, tag="zw")
                nc.sync.dma_start(
                    out=zw[:],
                    in_=wuz[:, (NU + j) * KD * P:(NU + j + 1) * KD * P])
                zps = [zpsp.tile([P, TH], f32, tag=f"zp{h}", name=f"zps{j}{h}")
                       for h in range(2)]
                for h in range(2):
                    for k in range(KD):
                        nc.tensor.matmul(
                            zps[h][:], zw[:, k * P:(k + 1) * P],
                            xt_sb[:, k * L + h * TH: k * L + (h + 1) * TH],
                            start=(k == 0), stop=(k == KD - 1))
                for h in range(2):
                    ysl = slice(j * L + h * TH, j * L + (h + 1) * TH)
                    if cfg["native_silu"]:
                        sz = tmp3.tile([P, TH], bf16, tag="sz")
                        nc.scalar.activation(sz[:], zps[h][:], AF.Silu)
                        nc.vector.tensor_mul(y2_sb[:, ysl], scr_sb[:, ysl],
                                             sz[:])
                    else:
                        sg = tmp3.tile([P, TH], f32, tag="sz")
                        t1 = tmp3.tile([P, TH], f32, tag="t1")
                        nc.scalar.activation(sg[:], zps[h][:], AF.Sigmoid)
                        nc.vector.tensor_mul(t1[:], scr_sb[:, ysl], sg[:])
                        nc.vector.tensor_mul(y2_sb[:, ysl], t1[:], zps[h][:])

    # ---------------- phase 4: out_proj -----------------------------
    with (
        tc.tile_pool(name="wop", bufs=2) as wop,
        tc.tile_pool(name="osbp", bufs=4) as osbp,
        tc.tile_pool(name="ops", bufs=2, space="PSUM") as ops,
    ):
        for m in range(KD):
            wo_sb = wop.tile([P, NJ * P], cfg["dt_wo"], tag="wo")
            nc.sync.dma_start(out=wo_sb[:],
                              in_=wo[:, m * NJ * P:(m + 1) * NJ * P])
            po = [ops.tile([P, TH], f32, tag=f"po{h}", name=f"po{h}")
                  for h in range(2)]
            for h in range(2):
                for k in range(NJ):
                    nc.tensor.matmul(
                        po[h][:], wo_sb[:, k * P:(k + 1) * P],
                        y2_sb[:, k * L + h * TH: k * L + (h + 1) * TH],
                        start=(k == 0), stop=(k == NJ - 1))
                osb = osbp.tile([P, TH], f32, tag="osb", name=f"osb{m}{h}")
                nc.scalar.activation(osb[:], po[h][:], AF.Copy)
                nc.sync.dma_start(
                    out=outT[m * P:(m + 1) * P, h * TH:(h + 1) * TH],
                    in_=osb[:])


def _np(x):
    return np.asarray(x, dtype=np.float32)


def _round_f32r(x):
    """Round-to-nearest-even at 11 mantissa bits (matches HW f32r)."""
    i = np.ascontiguousarray(x, np.float32).view(np.uint32).astype(np.uint64)
    shift = 23 - 11
    bias = ((i >> shift) & 1) + ((1 << (shift - 1)) - 1)
    return ((i + bias) >> shift << shift).astype(np.uint32).view(np.float32)


def pack_core(c, inp, cfg=CFG):
    """Build the input map for core c from the full-problem inputs."""
    d, b, half = c // 4, (c // 2) % 2, c % 2
    tag = "f" if d == 0 else "r"
    w_in = _np(inp["in_proj_w"])
    if d == 1:
        w_in = w_in[:, ::-1]
    w_u, w_z = w_in[:D_INNER], w_in[D_INNER:]
    conv_w = _np(inp[f"conv_w_{tag}"])[:, 0, :]          # [D_INNER, 4]
    conv_b = _np(inp[f"conv_b_{tag}"])
    x_proj_w = _np(inp[f"x_proj_w_{tag}"])               # [96, D_INNER]
    dt_w = _np(inp[f"dt_w_{tag}"])                       # [D_INNER, 64]
    dt_b = _np(inp[f"dt_b_{tag}"])
    a_full = -np.exp(_np(inp[f"A_log_{tag}"]))           # [D_INNER, 16]
    d_full = _np(inp[f"D_{tag}"])
    w_out = _np(inp["out_proj_w"])                       # [1024, D_INNER]

    x = _np(inp["x"])[b] * _np(inp["input_mask"])[b]     # [L, 1024]
    own = half * HALF
    oth = (1 - half) * HALF

    # u-tile channel order: own half first
    uch = [own + m * P for m in range(NJ)] + [oth + m * P for m in range(NJ)]

    wuz = np.empty((P, (NU + NJ) * KD * P), np.float32)
    convd = np.zeros((P, NU * D_CONV * P), np.float32)
    convb_a = np.empty((P, NU), np.float32)
    xpw_a = np.empty((P, NU * 96), np.float32)
    for m, ch in enumerate(uch):
        for k in range(KD):
            wuz[:, m * KD * P + k * P:(m * KD + k + 1) * P] = \
                w_u[ch:ch + P, k * P:(k + 1) * P].T
        for kk in range(D_CONV):
            blk = convd[:, (m * D_CONV + kk) * P:(m * D_CONV + kk + 1) * P]
            np.fill_diagonal(blk, conv_w[ch:ch + P, kk])
        convb_a[:, m] = conv_b[ch:ch + P]
        xpw_a[:, m * 96:(m + 1) * 96] = x_proj_w[:, ch:ch + P].T
    for j in range(NJ):
        ch = own + j * P
        for k in range(KD):
            wuz[:, ((NU + j) * KD + k) * P:((NU + j) * KD + k + 1) * P] = \
                w_z[ch:ch + P, k * P:(k + 1) * P].T

    dtw_a = np.empty((DT_RANK, NJ * P), np.float32)
    dtb_a = np.empty((P, NJ), np.float32)
    a_own = np.empty((P, NJ * D_STATE), np.float32)
    d_own = np.empty((P, NJ), np.float32)
    wo_a = np.empty((P, KD * NJ * P), np.float32)
    for j in range(NJ):
        ch = own + j * P
        dtw_a[:, j * P:(j + 1) * P] = dt_w[ch:ch + P, :].T
        dtb_a[:, j] = dt_b[ch:ch + P]
        a_own[:, j * D_STATE:(j + 1) * D_STATE] = a_full[ch:ch + P]
        d_own[:, j] = d_full[ch:ch + P]
    for m in range(KD):
        for k in range(NJ):
            wo_a[:, (m * NJ + k) * P:(m * NJ + k + 1) * P] = \
                w_out[m * P:(m + 1) * P, own + k * P: own + (k + 1) * P].T

    import ml_dtypes
    wo_cast = wo_a.astype(np.float32 if cfg["dt_wo"] == f32 else ml_dtypes.bfloat16)
    id_np = np.eye(P).astype(np.float32 if cfg["dt_ident"] == f32 else ml_dtypes.bfloat16)

    return dict(
        xT=_round_f32r(np.ascontiguousarray(x.T)),
        wuz=_round_f32r(wuz), convd=_round_f32r(convd),
        xpw=_round_f32r(xpw_a), dtw=_round_f32r(dtw_a), wo=wo_cast,
        a_own=a_own, d_own=d_own, convb=convb_a, dtb=dtb_a,
        ident=id_np,
    )


_NC_CACHE = {}


def _get_nc(repeat=1):
    if repeat not in _NC_CACHE:
        _NC_CACHE[repeat] = build_bass(CFG, repeat=repeat)
    return _NC_CACHE[repeat]


def kernel(**inputs):
    nc = _get_nc()
    in_maps = [pack_core(c, inputs) for c in range(8)]
    res = run_bass_kernel_spmd(nc, in_maps, core_ids=list(range(8)))
    out = np.zeros((B, L, D_MODEL), np.float32)
    for c in range(8):
        b = (c // 2) % 2
        out[b] += np.asarray(res.results[c]["outT"], np.float32).T
    return out



# revision 18
# speedup vs baseline: 1.0638x; 1.0431x over previous
"""BiMambaBlock Trainium2 kernel (Bass/Tile), 8-core SPMD.

Sharding: core c -> (direction d=c//4, batch b=(c//2)%2, channel-half h=c%2).
Each core computes the full conv'd/silu'd u (needed for the x_proj
contraction over all of d_inner), then runs the selective scan, gate and
out_proj for its 1024-channel half only.  The feature-axis flip of the
reverse direction is folded into the in_proj weight slices on the host.
Partial out_proj results (each summed over the core's channel half) are
added together on the host: out[b] = sum over the 4 cores (2 dirs x 2
halves) of that batch.

On-chip layout is transposed: features on partitions, time on the free
axis.  The scan is the native DVE tensor_tensor_scan (h = dA*h + dBu along
t) run per (128-channel tile, state) pair.  dA = exp(delta * A[:,n]) is a
single ScalarE activation with per-partition scale.  B/C time-rows are
replicated across partitions via DRAM-bounce broadcast DMAs.  The state
reduction sum_n C*h is PE identity-matmul accumulation in PSUM.

build_bass(cfg, repeat=R) unrolls the whole compute body R times (inputs
loaded once) so true device time can be measured as (T(R)-T(1))/(R-1),
cancelling the multi-ms PJRT-over-axon dispatch overhead.
"""

import numpy as np

import concourse.bacc as bacc
import concourse.mybir as mybir
from concourse.bass_utils import run_bass_kernel_spmd
from concourse.tile import TileContext

f32 = mybir.dt.float32
f32r = mybir.dt.float32r
bf16 = mybir.dt.bfloat16
AF = mybir.ActivationFunctionType
OP = mybir.AluOpType

D_MODEL = 1024
D_INNER = 2048
HALF = D_INNER // 2          # channels per core
D_STATE = 16
D_CONV = 4
DT_RANK = 64
L = 1024
B = 2
TH = 512                     # matmul free-dim tile (time half)
P = 128
KD = D_MODEL // P            # 8 k-tiles over d_model
NU = D_INNER // P            # 16 u d-tiles (full)
NJ = HALF // P               # 8 own d-tiles
JBS = [2, 2, 2, 2]           # j-blocks (PSUM: 2j * 2h y banks = 4)

CFG = dict(
    dt_bc=bf16,       # B/C replicated rows
    dt_du=bf16,       # delta*u
    dt_da=bf16,       # exp(delta*A)
    dt_dbu=bf16,      # du*B
    dt_h=bf16,        # scan state output
    dt_hc=bf16,       # h*C
    dt_y2=bf16,       # gated output (out_proj rhs)
    dt_ident=bf16,    # identity lhsT for state reduction (match dt_hc)
    dt_wo=bf16,       # out_proj weights (match dt_y2)
    gp_hc=False,
    native_silu=True,
    native_softplus=False,
    pair_split=False,  # halve phase-1: own channel half only + AllReduce x_dbl
)


def build_bass(cfg=CFG, repeat=1):
    nc = bacc.Bacc(enable_partition_id=False,
                   num_devices=8 if cfg.get("pair_split") else None)

    xT = nc.declare_dram_parameter("xT", [D_MODEL, L], f32r, isOutput=False)
    wuz = nc.declare_dram_parameter("wuz", [P, (NU + NJ) * KD * P], f32r, isOutput=False)
    convd = nc.declare_dram_parameter("convd", [P, NU * D_CONV * P], f32r, isOutput=False)
    xpw = nc.declare_dram_parameter("xpw", [P, NU * 96], f32r, isOutput=False)
    dtw = nc.declare_dram_parameter("dtw", [DT_RANK, NJ * P], f32r, isOutput=False)
    wo = nc.declare_dram_parameter("wo", [P, KD * NJ * P], cfg["dt_wo"], isOutput=False)
    a_own = nc.declare_dram_parameter("a_own", [P, NJ * D_STATE], f32, isOutput=False)
    d_own = nc.declare_dram_parameter("d_own", [P, NJ], f32, isOutput=False)
    convb = nc.declare_dram_parameter("convb", [P, NU], f32, isOutput=False)
    dtb = nc.declare_dram_parameter("dtb", [P, NJ], f32, isOutput=False)
    ident_in = nc.declare_dram_parameter("ident", [P, P], cfg["dt_ident"], isOutput=False)
    outT = nc.declare_dram_parameter("outT", [D_MODEL, L], f32, isOutput=True)

    with TileContext(nc) as tc:
        with (
            tc.tile_pool(name="res", bufs=1) as res,
            tc.tile_pool(name="dramp", bufs=2, space="DRAM") as dramp,
        ):
            # resident tensors
            xt_sb = res.tile([P, KD * L], f32r, tag="xt")
            for k in range(KD):
                nc.sync.dma_start(out=xt_sb[:, k * L:(k + 1) * L],
                                  in_=xT[k * P:(k + 1) * P, :])
            id_sb = res.tile([P, P], cfg["dt_ident"], tag="id")
            nc.sync.dma_start(out=id_sb[:], in_=ident_in[:])
            a_sb = res.tile([P, NJ * D_STATE], f32, tag="a")
            nc.sync.dma_start(out=a_sb[:], in_=a_own[:])
            d_sb = res.tile([P, NJ], f32, tag="d")
            nc.sync.dma_start(out=d_sb[:], in_=d_own[:])
            cb_sb = res.tile([P, NU], f32, tag="cb")
            nc.sync.dma_start(out=cb_sb[:], in_=convb[:])
            dtb_sb = res.tile([P, NJ], f32, tag="dtb")
            nc.sync.dma_start(out=dtb_sb[:], in_=dtb[:])
            dtw_sb = res.tile([DT_RANK, NJ * P], f32r, tag="dtw")
            nc.sync.dma_start(out=dtw_sb[:], in_=dtw[:])
            xpw_sb = res.tile([P, NU * 96], f32r, tag="xpw")
            nc.sync.dma_start(out=xpw_sb[:], in_=xpw[:])
            u_own = res.tile([P, NJ * L], bf16, tag="uown")
            y2_sb = res.tile([P, NJ * L], cfg["dt_y2"], tag="y2")
            xdbl_sb = res.tile([96, L], f32r, tag="xdbl")
            delta_sb = res.tile([P, NJ * L], f32, tag="delta")
            scr_sb = res.tile([P, NJ * L], f32, tag="scr")

            for rep in range(repeat):
                _body(nc, tc, cfg, rep, wuz, convd, dtw, wo, outT,
                      xt_sb, id_sb, a_sb, d_sb, cb_sb, dtb_sb, dtw_sb, xpw_sb,
                      u_own, y2_sb, xdbl_sb, delta_sb, scr_sb, dramp)
    nc.compile()
    return nc


def _body(nc, tc, cfg, rep, wuz, convd, dtw, wo, outT,
          xt_sb, id_sb, a_sb, d_sb, cb_sb, dtb_sb, dtw_sb, xpw_sb,
          u_own, y2_sb, xdbl_sb, delta_sb, scr_sb, dramp):
    bc_dram = dramp.tile([2 * D_STATE, L], cfg["dt_bc"], tag="bc",
                         name=f"bc{rep}")
    # ---------------- phase 1: in_proj(u) + conv + silu + x_proj ----
    with (
        tc.tile_pool(name="xdblp", bufs=1, space="PSUM") as xdblp,
    ):
        xdbl_ps = [xdblp.tile([96, TH], f32, tag=f"xd{h}", name=f"xdbl_ps{h}")
                   for h in range(2)]
        nu_l = NJ if cfg.get("pair_split") else NU
        with (
            tc.tile_pool(name="wp", bufs=3) as wp,
            tc.tile_pool(name="cp", bufs=3) as cp,
            tc.tile_pool(name="upre", bufs=4) as upre_p,
            tc.tile_pool(name="up", bufs=3) as up,
            tc.tile_pool(name="tmp1", bufs=6) as tmp1,
            tc.tile_pool(name="mmps", bufs=3, space="PSUM") as mmps,
        ):
            for m in range(nu_l):
                w_sb = wp.tile([P, KD * P], f32r, tag="w")
                nc.sync.dma_start(out=w_sb[:],
                                  in_=wuz[:, m * KD * P:(m + 1) * KD * P])
                ps = [mmps.tile([P, TH], f32, tag=f"mm{h}", name=f"ps{h}")
                      for h in range(2)]
                for h in range(2):
                    for k in range(KD):
                        nc.tensor.matmul(
                            ps[h][:], w_sb[:, k * P:(k + 1) * P],
                            xt_sb[:, k * L + h * TH: k * L + (h + 1) * TH],
                            start=(k == 0), stop=(k == KD - 1))
                upre = upre_p.tile([P, L + D_CONV - 1], f32r, tag="upre")
                nc.vector.memset(upre[:, 0:D_CONV - 1].bitcast(f32), 0.0)
                for h in range(2):
                    nc.scalar.activation(
                        upre[:, D_CONV - 1 + h * TH: D_CONV - 1 + (h + 1) * TH],
                        ps[h][:], AF.Copy)
                cd = cp.tile([P, D_CONV * P], f32r, tag="cd")
                nc.sync.dma_start(
                    out=cd[:], in_=convd[:, m * D_CONV * P:(m + 1) * D_CONV * P])
                psc = [mmps.tile([P, TH], f32, tag=f"mm{h}", name=f"psc{h}")
                       for h in range(2)]
                for h in range(2):
                    for kk in range(D_CONV):
                        nc.tensor.matmul(
                            psc[h][:], cd[:, kk * P:(kk + 1) * P],
                            upre[:, kk + h * TH: kk + h * TH + TH],
                            start=(kk == 0), stop=(kk == D_CONV - 1))
                u_m = up.tile([P, L], f32r, tag="u")
                for h in range(2):
                    sl = slice(h * TH, (h + 1) * TH)
                    if cfg["native_silu"]:
                        nc.scalar.activation(u_m[:, sl], psc[h][:], AF.Silu,
                                             bias=cb_sb[:, m:m + 1])
                    else:
                        sg = tmp1.tile([P, TH], f32, tag="sg")
                        uc = tmp1.tile([P, TH], f32, tag="uc")
                        nc.scalar.activation(sg[:], psc[h][:], AF.Sigmoid,
                                             bias=cb_sb[:, m:m + 1])
                        nc.scalar.activation(uc[:], psc[h][:], AF.Identity,
                                             bias=cb_sb[:, m:m + 1])
                        nc.vector.tensor_mul(u_m[:, sl], uc[:], sg[:])
                    nc.tensor.matmul(
                        xdbl_ps[h][:], xpw_sb[:, m * 96:(m + 1) * 96],
                        u_m[:, sl], start=(m == 0), stop=(m == nu_l - 1))
                if m < NJ:  # own half: keep bf16 copy for du and skip path
                    nc.scalar.activation(u_own[:, m * L:(m + 1) * L], u_m[:],
                                         AF.Copy)

        # ---------------- phase 2: drain x_dbl, stage B/C --------------
        if cfg.get("pair_split"):
            # partial x_dbl -> DRAM -> AllReduce(add) with pair core ->
            # back into xdbl_sb.
            xp_dram = dramp.tile([96, L], f32, tag="xp", name=f"xp{rep}")
            xr_dram = dramp.tile([96, L], f32, tag="xr", name=f"xr{rep}")
            with tc.tile_pool(name="xpart", bufs=1) as xpp:
                xpart = xpp.tile([96, L], f32, tag="xpart")
                for h in range(2):
                    nc.vector.tensor_copy(xpart[:, h * TH:(h + 1) * TH],
                                          xdbl_ps[h][:])
                nc.sync.dma_start(out=xp_dram[:], in_=xpart[:])
            nc.gpsimd.collective_compute(
                "AllReduce", OP.add,
                replica_groups=[[0, 1], [2, 3], [4, 5], [6, 7]],
                ins=[xp_dram[:].opt()], outs=[xr_dram[:].opt()])
            nc.sync.dma_start(out=xdbl_sb[:].bitcast(f32), in_=xr_dram[:])
        else:
            for h in range(2):
                nc.vector.tensor_copy(xdbl_sb[:, h * TH:(h + 1) * TH],
                                      xdbl_ps[h][:])
        if cfg["dt_bc"] == f32:
            nc.sync.dma_start(out=bc_dram[:], in_=xdbl_sb[DT_RANK:96, :])
        else:
            with tc.tile_pool(name="bcc", bufs=1) as bcc:
                bc_cast = bcc.tile([2 * D_STATE, L], cfg["dt_bc"], tag="bcc")
                nc.vector.tensor_copy(bc_cast[:], xdbl_sb[DT_RANK:96, :])
                nc.sync.dma_start(out=bc_dram[:], in_=bc_cast[:])

    # ---------------- phase 3: scan (delta/z/gate interleaved) ------
    # Per j-block: the block's delta (dt-proj + softplus) was computed at
    # the previous block boundary (or in the prologue for block 0), so the
    # Act engine computes the NEXT block's delta and this block's z-gate
    # while the DVE chews through the scan.  PSUM: 4 y + 2 z + 2 dt = 8.
    def _delta_block(dtps, js):
        for j in js:
            for h in range(2):
                dps = dtps.tile([P, TH], f32, tag="dt", name=f"dps{j}{h}")
                nc.tensor.matmul(
                    dps[:], dtw_sb[:, j * P:(j + 1) * P],
                    xdbl_sb[0:DT_RANK, h * TH:(h + 1) * TH],
                    start=True, stop=True)
                sl = slice(j * L + h * TH, j * L + (h + 1) * TH)
                if cfg["native_softplus"]:
                    nc.scalar.activation(delta_sb[:, sl], dps[:],
                                         AF.Softplus, bias=dtb_sb[:, j:j + 1])
                else:
                    nc.scalar.activation(scr_sb[:, sl], dps[:], AF.Exp,
                                         bias=dtb_sb[:, j:j + 1])
        if not cfg["native_softplus"]:
            for j in js:
                sl = slice(j * L, (j + 1) * L)
                nc.scalar.activation(delta_sb[:, sl], scr_sb[:, sl], AF.Ln,
                                     bias=1.0)

    with (
        tc.tile_pool(name="dup", bufs=3) as dup,
        tc.tile_pool(name="bcp", bufs=3) as bcp,
        tc.tile_pool(name="sda", bufs=3) as sda,
        tc.tile_pool(name="sdb", bufs=3) as sdb,
        tc.tile_pool(name="shp", bufs=3) as shp,
        tc.tile_pool(name="shc", bufs=3) as shc,
        tc.tile_pool(name="tmp3", bufs=4) as tmp3,
        tc.tile_pool(name="wzp", bufs=2) as wzp,
        tc.tile_pool(name="dtps", bufs=2, space="PSUM") as dtps,
        tc.tile_pool(name="yps", bufs=1, space="PSUM") as yps,
        tc.tile_pool(name="zpsp", bufs=1, space="PSUM") as zpsp,
    ):
        blocks = []
        j0 = 0
        for jb in JBS:
            blocks.append(list(range(j0, j0 + jb)))
            j0 += jb
        _delta_block(dtps, blocks[0])

        for b, js in enumerate(blocks):
            dus = {}
            for j in js:
                du_j = dup.tile([P, L], cfg["dt_du"], tag="du")
                nc.vector.tensor_mul(du_j[:], delta_sb[:, j * L:(j + 1) * L],
                                     u_own[:, j * L:(j + 1) * L])
                dus[j] = du_j

            y_ps = {j: [yps.tile([P, TH], f32, tag=f"y{j - js[0]}{h}",
                                 name=f"yps{j}{h}")
                        for h in range(2)] for j in js}
            for n in range(D_STATE):
                brep = bcp.tile([P, L], cfg["dt_bc"], tag="brep")
                nc.sync.dma_start(out=brep[:],
                                  in_=bc_dram[n, :].partition_broadcast(P))
                crep = bcp.tile([P, L], cfg["dt_bc"], tag="crep")
                nc.sync.dma_start(
                    out=crep[:],
                    in_=bc_dram[D_STATE + n, :].partition_broadcast(P))
                for j in js:
                    da = sda.tile([P, L], cfg["dt_da"], tag="da")
                    nc.scalar.activation(
                        da[:], delta_sb[:, j * L:(j + 1) * L], AF.Exp,
                        scale=a_sb[:, j * D_STATE + n: j * D_STATE + n + 1])
                    dbu = sdb.tile([P, L], cfg["dt_dbu"], tag="dbu")
                    nc.vector.tensor_mul(dbu[:], dus[j][:], brep[:])
                    h_t = shp.tile([P, L], cfg["dt_h"], tag="h")
                    nc.vector.tensor_tensor_scan(h_t[:], da[:], dbu[:], 0.0,
                                                 OP.mult, OP.add)
                    hc = shc.tile([P, L], cfg["dt_hc"], tag="hc")
                    nc.vector.tensor_mul(hc[:], h_t[:], crep[:])
                    for h in range(2):
                        nc.tensor.matmul(
                            y_ps[j][h][:], id_sb[:],
                            hc[:, h * TH:(h + 1) * TH],
                            start=(n == 0), stop=(n == D_STATE - 1))
                if n == 0 and b + 1 < len(blocks):
                    _delta_block(dtps, blocks[b + 1])

            for j in js:
                # y1 = u_own * D + y  -> scr_sb (exp scratch is free now)
                for h in range(2):
                    nc.vector.scalar_tensor_tensor(
                        scr_sb[:, j * L + h * TH: j * L + (h + 1) * TH],
                        u_own[:, j * L + h * TH: j * L + (h + 1) * TH],
                        d_sb[:, j:j + 1], y_ps[j][h][:], OP.mult, OP.add)
                # z-proj + gate: y2 = y1 * silu(z)
                zw = wzp.tile([P, KD * P], f32r, tag="zw")
                nc.sync.dma_start(
                    out=zw[:],
                    in_=wuz[:, (NU + j) * KD * P:(NU + j + 1) * KD * P])
                zps = [zpsp.tile([P, TH], f32, tag=f"zp{h}", name=f"zps{j}{h}")
                       for h in range(2)]
                for h in range(2):
                    for k in range(KD):
                        nc.tensor.matmul(
                            zps[h][:], zw[:, k * P:(k + 1) * P],
                            xt_sb[:, k * L + h * TH: k * L + (h + 1) * TH],
                            start=(k == 0), stop=(k == KD - 1))
                for h in range(2):
                    ysl = slice(j * L + h * TH, j * L + (h + 1) * TH)
                    if cfg["native_silu"]:
                        sz = tmp3.tile([P, TH], f32, tag="sz")
                        nc.scalar.activation(sz[:], zps[h][:], AF.Silu)
                        nc.vector.tensor_mul(y2_sb[:, ysl], scr_sb[:, ysl],
                                             sz[:])
                    else:
                        sg = tmp3.tile([P, TH], f32, tag="sz")
                        t1 = tmp3.tile([P, TH], f32, tag="t1")
                        nc.scalar.activation(sg[:], zps[h][:], AF.Sigmoid)
                        nc.vector.tensor_mul(t1[:], scr_sb[:, ysl], sg[:])
                        nc.vector.tensor_mul(y2_sb[:, ysl], t1[:], zps[h][:])

    # ---------------- phase 4: out_proj -----------------------------
    with (
        tc.tile_pool(name="wop", bufs=2) as wop,
        tc.tile_pool(name="osbp", bufs=4) as osbp,
        tc.tile_pool(name="ops", bufs=2, space="PSUM") as ops,
    ):
        for m in range(KD):
            wo_sb = wop.tile([P, NJ * P], cfg["dt_wo"], tag="wo")
            nc.sync.dma_start(out=wo_sb[:],
                              in_=wo[:, m * NJ * P:(m + 1) * NJ * P])
            po = [ops.tile([P, TH], f32, tag=f"po{h}", name=f"po{h}")
                  for h in range(2)]
            for h in range(2):
                for k in range(NJ):
                    nc.tensor.matmul(
                        po[h][:], wo_sb[:, k * P:(k + 1) * P],
                        y2_sb[:, k * L + h * TH: k * L + (h + 1) * TH],
                        start=(k == 0), stop=(k == NJ - 1))
                osb = osbp.tile([P, TH], f32, tag="osb", name=f"osb{m}{h}")
                nc.scalar.activation(osb[:], po[h][:], AF.Copy)
                nc.sync.dma_start(
                    out=outT[m * P:(m + 1) * P, h * TH:(h + 1) * TH],
                    in_=osb[:])


def _np(x):
    return np.asarray(x, dtype=np.float32)


def _round_f32r(x):
    """Round-to-nearest-even at 11 mantissa bits (matches HW f32r)."""
    i = np.ascontiguousarray(x, np.float32).view(np.uint32).astype(np.uint64)
    shift = 23 - 11
    bias = ((i >> shift) & 1) + ((1 << (shift - 1)) - 1)
    return ((i + bias) >> shift << shift).astype(np.uint32).view(np.float32)


def pack_core(c, inp, cfg=CFG):
    """Build the input map for core c from the full-problem inputs."""
    d, b, half = c // 4, (c // 2) % 2, c % 2
    tag = "f" if d == 0 else "r"
    w_in = _np(inp["in_proj_w"])
    if d == 1:
        w_in = w_in[:, ::-1]
    w_u, w_z = w_in[:D_INNER], w_in[D_INNER:]
    conv_w = _np(inp[f"conv_w_{tag}"])[:, 0, :]          # [D_INNER, 4]
    conv_b = _np(inp[f"conv_b_{tag}"])
    x_proj_w = _np(inp[f"x_proj_w_{tag}"])               # [96, D_INNER]
    dt_w = _np(inp[f"dt_w_{tag}"])                       # [D_INNER, 64]
    dt_b = _np(inp[f"dt_b_{tag}"])
    a_full = -np.exp(_np(inp[f"A_log_{tag}"]))           # [D_INNER, 16]
    d_full = _np(inp[f"D_{tag}"])
    w_out = _np(inp["out_proj_w"])                       # [1024, D_INNER]

    x = _np(inp["x"])[b] * _np(inp["input_mask"])[b]     # [L, 1024]
    own = half * HALF
    oth = (1 - half) * HALF

    # u-tile channel order: own half first
    uch = [own + m * P for m in range(NJ)] + [oth + m * P for m in range(NJ)]

    wuz = np.empty((P, (NU + NJ) * KD * P), np.float32)
    convd = np.zeros((P, NU * D_CONV * P), np.float32)
    convb_a = np.empty((P, NU), np.float32)
    xpw_a = np.empty((P, NU * 96), np.float32)
    for m, ch in enumerate(uch):
        for k in range(KD):
            wuz[:, m * KD * P + k * P:(m * KD + k + 1) * P] = \
                w_u[ch:ch + P, k * P:(k + 1) * P].T
        for kk in range(D_CONV):
            blk = convd[:, (m * D_CONV + kk) * P:(m * D_CONV + kk + 1) * P]
            np.fill_diagonal(blk, conv_w[ch:ch + P, kk])
        convb_a[:, m] = conv_b[ch:ch + P]
        xpw_a[:, m * 96:(m + 1) * 96] = x_proj_w[:, ch:ch + P].T
    for j in range(NJ):
        ch = own + j * P
        for k in range(KD):
            wuz[:, ((NU + j) * KD + k) * P:((NU + j) * KD + k + 1) * P] = \
                w_z[ch:ch + P, k * P:(k + 1) * P].T

    dtw_a = np.empty((DT_RANK, NJ * P), np.float32)
    dtb_a = np.empty((P, NJ), np.float32)
    a_own = np.empty((P, NJ * D_STATE), np.float32)
    d_own = np.empty((P, NJ), np.float32)
    wo_a = np.empty((P, KD * NJ * P), np.float32)
    for j in range(NJ):
        ch = own + j * P
        dtw_a[:, j * P:(j + 1) * P] = dt_w[ch:ch + P, :].T
        dtb_a[:, j] = dt_b[ch:ch + P]
        a_own[:, j * D_STATE:(j + 1) * D_STATE] = a_full[ch:ch + P]
        d_own[:, j] = d_full[ch:ch + P]
    for m in range(KD):
        for k in range(NJ):
            wo_a[:, (m * NJ + k) * P:(m * NJ + k + 1) * P] = \
                w_out[m * P:(m + 1) * P, own + k * P: own + (k + 1) * P].T

    import ml_dtypes
    wo_cast = wo_a.astype(np.float32 if cfg["dt_wo"] == f32 else ml_dtypes.bfloat16)
    id_np = np.eye(P).astype(np.float32 if cfg["dt_ident"] == f32 else ml_dtypes.bfloat16)

    return dict(
        xT=_round_f32r(np.ascontiguousarray(x.T)),
        wuz=_round_f32r(wuz), convd=_round_f32r(convd),
        xpw=_round_f32r(xpw_a), dtw=_round_f32r(dtw_a), wo=wo_cast,
        a_own=a_own, d_own=d_own, convb=convb_a, dtb=dtb_a,
        ident=id_np,
    )


_NC_CACHE = {}


def _get_nc(repeat=1):
    if repeat not in _NC_CACHE:
        _NC_CACHE[repeat] = build_bass(CFG, repeat=repeat)
    return _NC_CACHE[repeat]


def kernel(**inputs):
    nc = _get_nc()
    in_maps = [pack_core(c, inputs) for c in range(8)]
    res = run_bass_kernel_spmd(nc, in_maps, core_ids=list(range(8)))
    out = np.zeros((B, L, D_MODEL), np.float32)
    for c in range(8):
        b = (c // 2) % 2
        out[b] += np.asarray(res.results[c]["outT"], np.float32).T
    return out

